# revision 1
# baseline (speedup 1.0000x reference)
"""Trainium2 Bass kernel for nn_ClementsBellNxN (N=512, 8 cores).

Sharding: column-wise, 64 columns per core; zero communication.

Algorithm (per core, per step i of 256):
  even half-step: fused operator E_k = Mmi@diag(e^{i pa[2k]},e^{i pa[2k+1]})@Mmi
     applied to row pairs (2k, 2k+1); 2x2 symmetric complex [[a,b],[b,d]].
  odd half-step:  same with pb on pairs (2k+1, 2k+2); edge rows 0/511 get pure
     phase rotations, absorbed into spare coefficient lanes.

Storage: pair k -> (partition p=k//2, free-block b=k%2); tiles T(even rows)/
U(odd rows) split into 8 channels [128,64]: {T,U} x {re,im} x {b0,b1}.
The odd half's "odd-k" range needs t_{k+1} = T[p+1, b0]: partition shifts are
illegal in engine APs, so the shift runs on the idle TensorEngine via constant
permutation matmuls (Pfwd/Pbwd), with corner lanes carrying the edge rows.

Per 128-lane half-block the 2x2 apply uses the beta-symmetry trick (m=b*(t+u))
with a runtime-registered custom DVE op CMUL_SUB_ANT (out = C0*Src0 - C1*Src1,
per-partition scalar columns) fusing each complex rotation into one DVE
instruction. Engine split: DVE fused rotations + PSUM-adjacent STT chains,
GPSIMD the tensor-adds, ScalarE the tsh PSUM->SBUF copies, PE the shifts.
Coefficients pack 9 columns per (step, half, range): br,bi,-br, ar,ai,-ar,
dr,di,-dr (a := alpha-beta, d := delta-beta).
"""
import numpy as np

N = 512
S = 256
NCORES = 8
COLS = N // NCORES  # 64
IL = 0.05
IMB = 0.005
_sq = np.sqrt(1.0 - IL)
A = np.float64(np.float32(_sq * np.sqrt(0.5 + IMB)))
B = np.float64(np.float32(_sq * np.sqrt(0.5 - IMB)))

# ---------------------------------------------------------------- host math

def _fused2x2(ph_first, ph_second):
    p = np.exp(1j * np.float64(ph_first))
    q = np.exp(1j * np.float64(ph_second))
    alpha = A * A * p - B * B * q
    beta = 1j * A * B * (p + q)
    delta = A * A * q - B * B * p
    return alpha, beta, delta


def _pack6(dst, aa, bb, dd):
    amb, dmb = aa - bb, dd - bb
    dst[:, 0] = bb.real
    dst[:, 1] = bb.imag
    dst[:, 2] = -bb.real
    dst[:, 3] = amb.real
    dst[:, 4] = amb.imag
    dst[:, 5] = -amb.real
    dst[:, 6] = dmb.real
    dst[:, 7] = dmb.imag
    dst[:, 8] = -dmb.real


def _precompute(phases, nsteps):
    ph = np.float64(phases)
    k = np.arange(256)
    j = np.arange(128)
    ceven = np.zeros((128, nsteps, 2, 9), np.float64)
    codd = np.zeros((128, nsteps, 2, 9), np.float64)
    for i in range(nsteps):
        pa = ph[1 + 2 * i]
        pb = ph[2 + 2 * i]
        al, be, de = _fused2x2(pa[2 * k], pa[2 * k + 1])
        for b in range(2):
            sel = 2 * j + b
            _pack6(ceven[:, i, b], al[sel], be[sel], de[sel])
        ko = np.arange(255)
        alo, beo, deo = _fused2x2(pb[2 * ko + 1], pb[2 * ko + 2])
        alo = np.concatenate([alo, [0.0 + 0j]])
        beo = np.concatenate([beo, [0.0 + 0j]])
        deo = np.concatenate([deo, [0.0 + 0j]])
        _pack6(codd[:, i, 0], alo[2 * j], beo[2 * j], deo[2 * j])
        sel1 = np.minimum(2 * j + 1, 255)
        a1, b1_, d1 = alo[sel1].copy(), beo[sel1].copy(), deo[sel1].copy()
        a1[127] = np.exp(1j * pb[511])   # row 511 rotation (u-channel)
        b1_[127] = 0.0
        d1[127] = np.exp(1j * pb[0])     # row 0 rotation (t-channel via Pbwd)
        _pack6(codd[:, i, 1], a1, b1_, d1)
    p_ = np.arange(128)
    cfin = np.zeros((128, 8), np.float64)
    phf = ph[N + 1]
    for b in range(2):
        rT = 2 * (2 * p_ + b)
        cfin[:, 0 + b] = np.cos(phf[rT])
        cfin[:, 2 + b] = np.sin(phf[rT])
        cfin[:, 4 + b] = np.cos(phf[rT + 1])
        cfin[:, 6 + b] = np.sin(phf[rT + 1])
    pfwd = np.zeros((128, 128), np.float32)
    pfwd[np.arange(1, 128), np.arange(0, 127)] = 1.0
    pfwd[0, 127] = 1.0
    pbwd = np.zeros((128, 128), np.float32)
    pbwd[np.arange(0, 127), np.arange(1, 128)] = 1.0
    pbwd[127, 0] = 1.0
    return (ceven.reshape(128, nsteps * 18).astype(np.float32),
            codd.reshape(128, nsteps * 18).astype(np.float32),
            cfin.astype(np.float32), pfwd, pbwd)


def _initial_state(phases, col0, ncols):
    """Packed [128, 8*ncols] init: channels Tre0,Tre1,Tim0,Tim1,Ure0..Uim1."""
    ph0 = np.float64(phases[0])
    out = np.zeros((128, 8, ncols), np.float64)
    p = np.arange(128)
    for b in range(2):
        kk = 2 * p + b
        rt = 2 * kk
        ru = rt + 1
        mt = (rt >= col0) & (rt < col0 + ncols)
        mu = (ru >= col0) & (ru < col0 + ncols)
        out[p[mt], 0 + b, rt[mt] - col0] = np.cos(ph0[rt[mt]])
        out[p[mt], 2 + b, rt[mt] - col0] = np.sin(ph0[rt[mt]])
        out[p[mu], 4 + b, ru[mu] - col0] = np.cos(ph0[ru[mu]])
        out[p[mu], 6 + b, ru[mu] - col0] = np.sin(ph0[ru[mu]])
    return out.reshape(128, 8 * ncols).astype(np.float32)

# ---------------------------------------------------------------- bass build

_CACHE = {}
_CMUL = []


def _ensure_cmul_op():
    """Register a custom DVE op: out = C0*Src0 - C1*Src1 (per-partition
    scalars). One uop; sha self-pinned at registration."""
    if _CMUL:
        return _CMUL[0]
    import concourse.dve_ops as D
    from concourse.dve_spec import Src0, Src1, C0, C1, lower, _has_src1
    from concourse.dve_uop import DveOpSpec
    from concourse.dve_table_gen import dve_ver_for

    name = "CMUL_SUB_ANT"
    for o in D.OPS:
        if o.name == name:
            _CMUL.append(o)
            return o
    spec = D.Spec(body=(Src0 * C0) - (Src1 * C1), accum=None, accum_init=None,
                  reference=lambda in0, in1, c0, c1, c2: in0 * c0 - in1 * c1)
    ver = dve_ver_for("TRN2")
    opcode = 1 + len(D.OPS)
    tmp = DveOpSpec(name=name, opcode=opcode, uops=lower(spec, ver=ver),
                    rd1_en=_has_src1(spec))
    op = D.DveOp(name=name, spec=spec, subdim=False,
                 uops_sha={ver: tmp.sha(ver)})
    D.OPS.append(op)
    D._SUB_OPCODE_FOR_NAME[name] = opcode
    D.CUSTOM_DVE_SPECS[name] = spec
    _CMUL.append(op)
    return op


def _build(nsteps=S):
    import concourse.mybir as mybir
    from concourse import bacc, tile

    f32 = mybir.dt.float32
    add, sub, mul = (mybir.AluOpType.add, mybir.AluOpType.subtract,
                     mybir.AluOpType.mult)

    nc = bacc.Bacc("TRN2", target_bir_lowering=False, debug=False,
                   enable_asserts=False)
    ce_d = nc.dram_tensor("ceven", [128, nsteps * 18], f32, kind="ExternalInput")
    co_d = nc.dram_tensor("codd", [128, nsteps * 18], f32, kind="ExternalInput")
    cf_d = nc.dram_tensor("cfin", [128, 8], f32, kind="ExternalInput")
    pf_d = nc.dram_tensor("pfwd", [128, 128], f32, kind="ExternalInput")
    pb_d = nc.dram_tensor("pbwd", [128, 128], f32, kind="ExternalInput")
    in_d = nc.dram_tensor("init", [128, 8 * COLS], f32, kind="ExternalInput")
    out_d = nc.dram_tensor("out", [128, 8 * COLS], f32, kind="ExternalOutput")

    with tile.TileContext(nc) as tc:
        with (
            tc.tile_pool(name="coef", bufs=1) as cpool,
            tc.tile_pool(name="state", bufs=4) as spool,
            tc.tile_pool(name="tmp", bufs=8) as tpool,
            tc.tile_pool(name="psum", bufs=2, space="PSUM") as ppool,
        ):
            ce = cpool.tile([128, nsteps * 18], f32, tag="ce")
            co = cpool.tile([128, nsteps * 18], f32, tag="co")
            cf = cpool.tile([128, 8], f32, tag="cf")
            pf = cpool.tile([128, 128], f32, tag="pf")
            pb = cpool.tile([128, 128], f32, tag="pb")
            ini = cpool.tile([128, 8 * COLS], f32, tag="ini")
            obuf = cpool.tile([128, 8 * COLS], f32, tag="obuf")
            nc.sync.dma_start(out=ce[:], in_=ce_d.ap())
            nc.sync.dma_start(out=co[:], in_=co_d.ap())
            nc.sync.dma_start(out=cf[:], in_=cf_d.ap())
            nc.sync.dma_start(out=pf[:], in_=pf_d.ap())
            nc.sync.dma_start(out=pb[:], in_=pb_d.ap())
            nc.sync.dma_start(out=ini[:], in_=in_d.ap())

            # current state APs per channel: Tre0,Tre1,Tim0,Tim1,Ure0,Ure1,Uim0,Uim1
            cur = [ini[:, ch * COLS:(ch + 1) * COLS] for ch in range(8)]

            cmul_op = _ensure_cmul_op()

            def cmul(out, i0, i1, sc0, sc1):
                # out = sc0*i0 - sc1*i1  (per-partition scalar columns)
                nc.vector._custom_dve(cmul_op, out=out, in0=i0, in1=i1,
                                      s0=sc0, s1=sc1)

            def half_block(tre, tim, ure, uim, coef, cb, outs,
                           bt=False, bu=False, s_on_dve=False):
                """Apply [[a,b],[b,d]] to (t,u); coef cols cb..cb+9 =
                br,bi,nbr, ar,ai,nar, dr,di,ndr (n* = negated).
                outs = (otre, otim, oure, ouim) destination APs.
                s-adds: GPSIMD tensor_tensor (DVE STT when a PSUM input).
                m and scheme-B rotations: one fused CMUL_SUB_ANT DVE op each;
                scheme-B final adds on GPSIMD. bt/bu pick scheme B for the
                t/u output pair; scheme A = 2 chained DVE STTs (PSUM-safe,
                shortest path for the PE-coupled slots)."""
                br = coef[:, cb + 0:cb + 1]
                bi = coef[:, cb + 1:cb + 2]
                nbr = coef[:, cb + 2:cb + 3]
                otre, otim, oure, ouim = outs
                v = nc.vector
                g = nc.gpsimd
                s_re = tpool.tile([128, COLS], f32, tag="s_re")
                s_im = tpool.tile([128, COLS], f32, tag="s_im")
                m_re = tpool.tile([128, COLS], f32, tag="m_re")
                m_im = tpool.tile([128, COLS], f32, tag="m_im")
                if s_on_dve:
                    cmul(s_re[:], tre, ure, 1.0, -1.0)
                    cmul(s_im[:], tim, uim, 1.0, -1.0)
                else:
                    g.tensor_add(out=s_re[:], in0=tre, in1=ure)
                    g.tensor_add(out=s_im[:], in0=tim, in1=uim)
                # m = beta * s (complex)
                cmul(m_re[:], s_re[:], s_im[:], br, bi)
                cmul(m_im[:], s_re[:], s_im[:], bi, nbr)

                def out_pair(ore, oim, xre, xim, c0, scheme_b):
                    # ore = cr*xre - ci*xim + m_re ; oim = ci*xre + cr*xim + m_im
                    cr = coef[:, cb + c0:cb + c0 + 1]
                    ci = coef[:, cb + c0 + 1:cb + c0 + 2]
                    ncr = coef[:, cb + c0 + 2:cb + c0 + 3]
                    if scheme_b:
                        z1 = tpool.tile([128, COLS], f32, tag="z1")
                        z2 = tpool.tile([128, COLS], f32, tag="z2")
                        cmul(z1[:], xre, xim, cr, ci)
                        g.tensor_add(out=ore, in0=z1[:], in1=m_re[:])
                        cmul(z2[:], xre, xim, ci, ncr)
                        g.tensor_add(out=oim, in0=z2[:], in1=m_im[:])
                    else:
                        v.scalar_tensor_tensor(out=ore, in0=xim, scalar=ci,
                                               in1=m_re[:], op0=mul, op1=sub)
                        v.scalar_tensor_tensor(out=ore, in0=xre, scalar=cr,
                                               in1=ore, op0=mul, op1=sub)
                        v.scalar_tensor_tensor(out=oim, in0=xre, scalar=ci,
                                               in1=m_im[:], op0=mul, op1=add)
                        v.scalar_tensor_tensor(out=oim, in0=xim, scalar=cr,
                                               in1=oim, op0=mul, op1=add)

                out_pair(otre, otim, tre, tim, 3, bt)
                out_pair(oure, ouim, ure, uim, 6, bu)

            for i in range(nsteps):
                # ---------------- even half ----------------
                nxt = [spool.tile([128, COLS], f32, tag=f"st{ch}", name=f"st{ch}_{i}")
                       for ch in range(8)]
                for b in range(2):
                    cb = (i * 2 + b) * 9
                    half_block(cur[0 + b], cur[2 + b], cur[4 + b], cur[6 + b],
                               ce, cb,
                               (nxt[0 + b][:], nxt[2 + b][:],
                                nxt[4 + b][:], nxt[6 + b][:]),
                               bt=(b == 1), bu=True, s_on_dve=(b == 0))
                # ---------------- odd half -----------------
                nx2 = [spool.tile([128, COLS], f32, tag=f"so{ch}", name=f"so{ch}_{i}")
                       for ch in range(8)]
                # range 0 (even k): (u = U[:,b0], t = T[:,b1]) aligned
                cb = (i * 2 + 0) * 9
                half_block(nxt[4][:], nxt[6][:], nxt[1][:], nxt[3][:],
                           co, cb,
                           (nx2[4][:], nx2[6][:], nx2[1][:], nx2[3][:]),
                           bt=True, bu=True)
                # PE shift: tsh = Pfwd . T'[:, b0]
                tsh_re = ppool.tile([128, COLS], f32, tag="tshre")
                tsh_im = ppool.tile([128, COLS], f32, tag="tshim")
                nc.tensor.matmul(out=tsh_re[:], lhsT=pf[:], rhs=nxt[0][:],
                                 start=True, stop=True)
                nc.tensor.matmul(out=tsh_im[:], lhsT=pf[:], rhs=nxt[2][:],
                                 start=True, stop=True)
                tshs_re = spool.tile([128, COLS], f32, tag="tshsre",
                                     name=f"tshsre_{i}")
                tshs_im = spool.tile([128, COLS], f32, tag="tshsim",
                                     name=f"tshsim_{i}")
                nc.scalar.copy(tshs_re[:], tsh_re[:])
                nc.scalar.copy(tshs_im[:], tsh_im[:])
                # range 1 (odd k): (u = U[:,b1], t = tsh)
                tt_re = tpool.tile([128, COLS], f32, tag="tt_re")
                tt_im = tpool.tile([128, COLS], f32, tag="tt_im")
                cb = (i * 2 + 1) * 9
                half_block(nxt[5][:], nxt[7][:], tshs_re[:], tshs_im[:],
                           co, cb,
                           (nx2[5][:], nx2[7][:], tt_re[:], tt_im[:]),
                           bt=True, bu=False, s_on_dve=False)
                # PE shift back: T''[:, b0] = Pbwd . tt  (lands in PSUM)
                t0_re = ppool.tile([128, COLS], f32, tag="t0re")
                t0_im = ppool.tile([128, COLS], f32, tag="t0im")
                nc.tensor.matmul(out=t0_re[:], lhsT=pb[:], rhs=tt_re[:],
                                 start=True, stop=True)
                nc.tensor.matmul(out=t0_im[:], lhsT=pb[:], rhs=tt_im[:],
                                 start=True, stop=True)
                cur = [t0_re[:], nx2[1][:], t0_im[:], nx2[3][:],
                       nx2[4][:], nx2[5][:], nx2[6][:], nx2[7][:]]

            # ---------------- final rotation + store ----------------
            v = nc.vector
            for tile_i in range(2):      # T, U
                for b in range(2):
                    cosc = cf[:, 4 * tile_i + b:4 * tile_i + b + 1]
                    sinc = cf[:, 4 * tile_i + 2 + b:4 * tile_i + 2 + b + 1]
                    re = cur[4 * tile_i + b]
                    im = cur[4 * tile_i + 2 + b]
                    ore = obuf[:, (4 * tile_i + b) * COLS:
                               (4 * tile_i + b + 1) * COLS]
                    oim = obuf[:, (4 * tile_i + 2 + b) * COLS:
                               (4 * tile_i + 2 + b + 1) * COLS]
                    x = tpool.tile([128, COLS], f32, tag="fx")
                    y = tpool.tile([128, COLS], f32, tag="fy")
                    v.tensor_scalar_mul(out=x[:], in0=im, scalar1=sinc)
                    v.scalar_tensor_tensor(out=ore, in0=re, scalar=cosc,
                                           in1=x[:], op0=mul, op1=sub)
                    v.tensor_scalar_mul(out=y[:], in0=re, scalar1=sinc)
                    v.scalar_tensor_tensor(out=oim, in0=im, scalar=cosc,
                                           in1=y[:], op0=mul, op1=add)
            nc.sync.dma_start(out=out_d.ap(), in_=obuf[:])
    nc.compile()
    return nc


def _get_module(nsteps=S):
    if nsteps not in _CACHE:
        _CACHE[nsteps] = _build(nsteps)
    return _CACHE[nsteps]


# ---------------------------------------------------------------- entry

def kernel(phases: np.ndarray) -> np.ndarray:
    from concourse.bass_utils import run_bass_kernel_spmd

    phases = np.asarray(phases)
    nc = _get_module(S)
    ce, co, cfin, pfwd, pbwd = _precompute(phases, S)
    in_maps = []
    for c in range(NCORES):
        in_maps.append({
            "ceven": ce, "codd": co, "cfin": cfin,
            "pfwd": pfwd, "pbwd": pbwd,
            "init": _initial_state(phases, c * COLS, COLS),
        })
    res = run_bass_kernel_spmd(nc, in_maps, core_ids=list(range(NCORES)))
    M = np.zeros((N, N), np.complex64)
    p = np.arange(128)
    for c in range(NCORES):
        o = res.results[c]["out"].reshape(128, 8, COLS)
        cols = slice(c * COLS, (c + 1) * COLS)
        for b in range(2):
            M[2 * (2 * p + b), cols] = o[:, 0 + b] + 1j * o[:, 2 + b]
            M[2 * (2 * p + b) + 1, cols] = o[:, 4 + b] + 1j * o[:, 6 + b]
    return M



# revision 18
# speedup vs baseline: 22.0596x; 22.0596x over previous
"""Trainium2 Bass kernel for nn_ClementsBellNxN (N=512, 8 NeuronCores).

Decomposition: the 512 fused 2x2 layers (256 steps x [even, odd]) are split
into 8 groups of 64 layers. Each group's operator B_g is a banded matrix
(half-bandwidth 64). M = D_last . B_7 ... B_0 . D_0.

Launch 1 (SPMD, core g builds BT_g = B_g^T):
  Host fuses each run of 16 layers into a half-bandwidth-16 operator F_s
  (cheap banded numpy; ~4% of device flops). On device, core g computes
  V <- F_s^T . V for s = 4..1 starting from V = I (core 7: D_last; D_0 is
  folded into F_1 of core 0). Block-tridiagonal fp16 PE matmuls with f32
  PSUM accumulation; V stored as 4 row-blocks [128, 264] (absolute column
  windows) x {re, im}.

Host relay: gathers the 8 transposed bands (fp16), chops them into lhsT
  tiles for phase 2, builds each core's initial X = (B_0 D_0)[:, cols].

Launch 2 (SPMD, core c owns 64 columns): X <- B_g . X for g = 1..7 via the
  same block-tridiagonal fp16 PE matmuls (diag [128,128] + corner [64,64]
  triangles), f32 PSUM, fp16 X between groups, f32 out.

Insertion loss scaling: each fused layer carries norm factor 0.95; bands are
rescaled by 0.95^-1 per layer on host and the global 0.95^512 is applied to
the final output to keep everything in fp16 range.
"""
import numpy as np

N = 512
NCORES = 8
GROUPS = 8
SPG = 32          # steps per group
LPG = 64          # fused layers per group
L = 16            # layers per device-fused operator F
NF = LPG // L     # 4 F-operators per group
HB = L            # half-bandwidth of F
WO = 17           # skew window center for host F build
WF = 35           # skew window width for host F build
VW = 264          # on-device V tile column-window width
VB = 68           # V window: block i covers absolute cols [128i-VB, 128i-VB+VW)
COLS = N // NCORES

IL = 0.05
IMB = 0.005
_sq = np.sqrt(1.0 - IL)
A = np.float64(np.float32(_sq * np.sqrt(0.5 + IMB)))
B = np.float64(np.float32(_sq * np.sqrt(0.5 - IMB)))
SC = 1.0 / (1.0 - IL)          # per-fused-layer rescale (|.|^2 norm factor)
FINAL_SCALE = np.float64(1.0 - IL) ** 512

# ---------------------------------------------------------------- host math


def _fused2x2(p1, p2):
    p = np.exp(1j * p1)
    q = np.exp(1j * p2)
    al = A * A * p - B * B * q
    be = 1j * A * B * (p + q)
    de = A * A * q - B * B * p
    return al * SC, be * SC, de * SC


def _shift_m1(X):
    out = np.zeros_like(X)
    out[:, 1:] = X[:, :-1]
    return out


def _shift_p1(X):
    out = np.zeros_like(X)
    out[:, :-1] = X[:, 1:]
    return out


def _apply_even_skew(W, pa):
    # left-multiply skewed band W[r, d] (d = c - r + WO) by Efused(pa)
    k = np.arange(N // 2)
    al, be, de = _fused2x2(pa[2 * k], pa[2 * k + 1])
    T, U = W[0::2], W[1::2]
    nT = al[:, None] * T + be[:, None] * _shift_m1(U)
    nU = be[:, None] * _shift_p1(T) + de[:, None] * U
    W[0::2], W[1::2] = nT, nU


def _apply_odd_skew(W, pb):
    k = np.arange(N // 2 - 1)
    al, be, de = _fused2x2(pb[2 * k + 1], pb[2 * k + 2])
    T, U = W[1:510:2], W[2:511:2]
    nT = al[:, None] * T + be[:, None] * _shift_m1(U)
    nU = be[:, None] * _shift_p1(T) + de[:, None] * U
    W[1:510:2], W[2:511:2] = nT, nU
    W[0] *= np.exp(1j * pb[0]) * SC
    W[511] *= np.exp(1j * pb[511]) * SC


def _group_layer_phases(phases, g):
    """(kind, phase-row) for the 64 layers of group g in application order."""
    out = []
    for i in range(g * SPG, (g + 1) * SPG):
        out.append(('E', np.float64(phases[1 + 2 * i])))
        out.append(('O', np.float64(phases[2 + 2 * i])))
    return out


def _fused_F_dense(phases, g):
    """The NF=4 fused operators of group g as dense [512, 512] complex128."""
    layers = _group_layer_phases(phases, g)
    r = np.arange(N)
    Fs = []
    for s in range(NF):
        W = np.zeros((N, WF), np.complex128)
        W[:, WO] = 1.0
        for kind, ph in layers[s * L:(s + 1) * L]:
            if kind == 'E':
                _apply_even_skew(W, ph)
            else:
                _apply_odd_skew(W, ph)
        F = np.zeros((N, N), np.complex128)
        for dd in range(WF):
            off = dd - WO
            rr = r[(r + off >= 0) & (r + off < N)]
            F[rr, rr + off] = W[rr, dd]
        Fs.append(F)
    if g == 0:
        Fs[0] = Fs[0] * np.exp(1j * np.float64(phases[0]))[None, :]  # F.D0
    return Fs


def _pack_launch1_inputs(phases, g):
    """fc [128, NF*4*3*128], fs [16, NF*6*3*16], vinit [128, 4*2*VW] (f16)."""
    Fs = _fused_F_dense(phases, g)
    fc = np.zeros((128, NF * 4 * 3 * 128), np.float16)
    # side blocks, 64x64 (PE base partitions limited to {0, 32, 64}):
    # partitions 0:64 hold the "dn" block of boundary t (j=t+1 -> out t),
    # partitions 64:128 hold the "up" block (j=t -> out t+1).
    fs = np.zeros((128, NF * 3 * 3 * 64), np.float16)
    for s in range(NF):
        F = Fs[s]
        for i in range(4):
            blk = F[128 * i:128 * (i + 1), 128 * i:128 * (i + 1)]
            base = ((s * 4 + i) * 3) * 128
            fc[:, base:base + 128] = blk.real.astype(np.float16)
            fc[:, base + 128:base + 256] = blk.imag.astype(np.float16)
            fc[:, base + 256:base + 384] = (-blk.imag).astype(np.float16)
        for t in range(3):
            dn = F[128 * (t + 1):128 * (t + 1) + 64,
                   128 * (t + 1) - 64:128 * (t + 1)]
            up = F[128 * t + 64:128 * (t + 1),
                   128 * (t + 1):128 * (t + 1) + 64]
            base = ((s * 3 + t) * 3) * 64
            fs[0:64, base:base + 64] = dn.real.astype(np.float16)
            fs[0:64, base + 64:base + 128] = dn.imag.astype(np.float16)
            fs[0:64, base + 128:base + 192] = (-dn.imag).astype(np.float16)
            fs[64:128, base:base + 64] = up.real.astype(np.float16)
            fs[64:128, base + 64:base + 128] = up.imag.astype(np.float16)
            fs[64:128, base + 128:base + 192] = (-up.imag).astype(np.float16)
    vinit = np.zeros((128, 4 * 2 * VW), np.float16)
    p = np.arange(128)
    if g == GROUPS - 1:
        dr = np.cos(np.float64(phases[N + 1]))
        di = np.sin(np.float64(phases[N + 1]))
    else:
        dr = np.ones(N)
        di = np.zeros(N)
    for i in range(4):
        rows = 128 * i + p
        vinit[p, (i * 2 + 0) * VW + VB + p] = dr[rows].astype(np.float16)
        vinit[p, (i * 2 + 1) * VW + VB + p] = di[rows].astype(np.float16)
    return {"fc": fc, "fs": fs, "vinit": vinit}


def _unpack_band(vout):
    """vout [128, 4*2*VW] f16 -> BT dense ([512,512] f16 re, im)."""
    btr = np.zeros((N, N), np.float16)
    bti = np.zeros((N, N), np.float16)
    for i in range(4):
        lo = 128 * i - VB
        c0, c1 = max(0, lo), min(N, lo + VW)
        btr[128 * i:128 * (i + 1), c0:c1] = \
            vout[:, (i * 2 + 0) * VW + (c0 - lo):(i * 2 + 0) * VW + (c1 - lo)]
        bti[128 * i:128 * (i + 1), c0:c1] = \
            vout[:, (i * 2 + 1) * VW + (c0 - lo):(i * 2 + 1) * VW + (c1 - lo)]
    return btr, bti


def _pack_launch2_inputs(bts, core):
    """bands [128, 7*4*3*128] diag + bsides [64, 7*6*3*64] + xinit."""
    bd = np.zeros((128, (GROUPS - 1) * 4 * 3 * 128), np.float16)
    # side blocks, 64x64: partitions 0:64 = dn (j=t+1 -> out t), partitions
    # 64:128 = up (j=t -> out t+1). One 64-col slot per (group, boundary, var).
    bs = np.zeros((128, (GROUPS - 1) * 3 * 3 * 64), np.float16)
    for g in range(1, GROUPS):
        btr, bti = bts[g]
        gg = g - 1
        for i in range(4):
            base = ((gg * 4 + i) * 3) * 128
            r0 = 128 * i
            bd[:, base:base + 128] = btr[r0:r0 + 128, r0:r0 + 128]
            bd[:, base + 128:base + 256] = bti[r0:r0 + 128, r0:r0 + 128]
            bd[:, base + 256:base + 384] = -bti[r0:r0 + 128, r0:r0 + 128]
        for t in range(3):
            qd, md = 128 * (t + 1), 128 * t + 64     # dn: BT[qd:+64, md:+64]
            qu, mu = 128 * t + 64, 128 * (t + 1)     # up: BT[qu:+64, mu:+64]
            base = ((gg * 3 + t) * 3) * 64
            bs[0:64, base:base + 64] = btr[qd:qd + 64, md:md + 64]
            bs[0:64, base + 64:base + 128] = bti[qd:qd + 64, md:md + 64]
            bs[0:64, base + 128:base + 192] = -bti[qd:qd + 64, md:md + 64]
            bs[64:128, base:base + 64] = btr[qu:qu + 64, mu:mu + 64]
            bs[64:128, base + 64:base + 128] = bti[qu:qu + 64, mu:mu + 64]
            bs[64:128, base + 128:base + 192] = -bti[qu:qu + 64, mu:mu + 64]
    btr0, bti0 = bts[0]
    cols = slice(core * COLS, (core + 1) * COLS)
    xinit = np.zeros((128, 4 * 2 * COLS), np.float16)
    for i in range(4):
        xinit[:, (2 * i + 0) * COLS:(2 * i + 1) * COLS] = \
            btr0[cols, 128 * i:128 * (i + 1)].T
        xinit[:, (2 * i + 1) * COLS:(2 * i + 2) * COLS] = \
            bti0[cols, 128 * i:128 * (i + 1)].T
    return {"bands": bd, "bsides": bs, "xinit": xinit}


# ---------------------------------------------------------------- bass build

_CACHE = {}


def _build_launch1():
    import concourse.mybir as mybir
    from concourse import bacc, tile

    f16 = mybir.dt.float16
    f32 = mybir.dt.float32

    nc = bacc.Bacc("TRN2", target_bir_lowering=False, debug=False,
                   enable_asserts=False)
    fc_d = nc.dram_tensor("fc", [128, NF * 4 * 3 * 128], f16,
                          kind="ExternalInput")
    fs_d = nc.dram_tensor("fs", [128, NF * 3 * 3 * 64], f16,
                          kind="ExternalInput")
    vi_d = nc.dram_tensor("vinit", [128, 4 * 2 * VW], f16,
                          kind="ExternalInput")
    vo_d = nc.dram_tensor("vout", [128, 4 * 2 * VW], f16,
                          kind="ExternalOutput")

    with tile.TileContext(nc) as tc:
        with (
            tc.tile_pool(name="coef", bufs=1) as cpool,
            tc.tile_pool(name="state", bufs=1) as spool,
            tc.tile_pool(name="psum", bufs=1, space="PSUM") as ppool,
        ):
            fc = cpool.tile([128, NF * 4 * 3 * 128], f16, tag="fc")
            fs = cpool.tile([128, NF * 3 * 3 * 64], f16, tag="fs")
            va = spool.tile([128, 4 * 2 * VW], f16, tag="va")
            vb = spool.tile([128, 4 * 2 * VW], f16, tag="vb")
            nc.sync.dma_start(out=fc[:], in_=fc_d.ap())
            nc.sync.dma_start(out=fs[:], in_=fs_d.ap())
            nc.sync.dma_start(out=va[:], in_=vi_d.ap())

            def FC(s, i, var):
                base = ((s * 4 + i) * 3 + var) * 128
                return fc[:, base:base + 128]

            def FS(s, t, var):
                base = ((s * 3 + t) * 3 + var) * 64
                return fs[:, base:base + 64]

            def VCH(t, i, comp):
                return t[:, (i * 2 + comp) * VW:(i * 2 + comp + 1) * VW]

            R, I, In = 0, 1, 2
            bufs = [va, vb]
            for sidx, s in enumerate(reversed(range(NF))):
                src, dst = bufs[sidx % 2], bufs[(sidx + 1) % 2]
                for i in range(4):
                    for comp in range(2):
                        ps = ppool.tile([128, VW], f32, tag=f"ps{i}_{comp}",
                                        name=f"ps{i}_{comp}_{s}")
                        # central: out_R = FR.Vr - FI.Vi ; out_I = FR.Vi+FI.Vr
                        m1 = (R, 0) if comp == 0 else (R, 1)
                        m2 = (In, 1) if comp == 0 else (I, 0)
                        nc.tensor.matmul(
                            out=ps[:], lhsT=FC(s, i, m1[0]),
                            rhs=VCH(src, i, m1[1]), start=True, stop=False,
                            skip_group_check=True)
                        nc.tensor.matmul(
                            out=ps[:], lhsT=FC(s, i, m2[0]),
                            rhs=VCH(src, i, m2[1]), start=False, stop=False,
                            skip_group_check=True)
                        mms = []
                        if i < 3:   # dn side: j = i+1, boundary t = i
                            for (var, c_in) in ([(R, 0), (In, 1)] if comp == 0
                                                else [(R, 1), (I, 0)]):
                                mms.append((FS(s, i, var)[0:64, :],
                                            VCH(src, i + 1, c_in)[0:64, 0:136],
                                            ps[64:128, 128:264]))
                        if i > 0:   # up side: j = i-1, boundary t = i-1
                            for (var, c_in) in ([(R, 0), (In, 1)] if comp == 0
                                                else [(R, 1), (I, 0)]):
                                mms.append((FS(s, i - 1, var)[64:128, :],
                                            VCH(src, i - 1, c_in)[64:128,
                                                                  128:264],
                                            ps[0:64, 0:136]))
                        for k, (lh, rh, po) in enumerate(mms):
                            nc.tensor.matmul(out=po, lhsT=lh, rhs=rh,
                                             start=False,
                                             stop=(k == len(mms) - 1),
                                             skip_group_check=True)
                        # evacuate PSUM -> dst (f16)
                        if (i * 2 + comp) % 2 == 0:
                            nc.vector.tensor_copy(out=VCH(dst, i, comp),
                                                  in_=ps[:])
                        else:
                            nc.scalar.copy(VCH(dst, i, comp), ps[:])
            final = bufs[NF % 2]
            nc.sync.dma_start(out=vo_d.ap(), in_=final[:])
    nc.compile()
    return nc


def _build_launch2():
    import concourse.mybir as mybir
    from concourse import bacc, tile

    f16 = mybir.dt.float16
    f32 = mybir.dt.float32

    nc = bacc.Bacc("TRN2", target_bir_lowering=False, debug=False,
                   enable_asserts=False)
    bd_d = nc.dram_tensor("bands", [128, (GROUPS - 1) * 4 * 3 * 128], f16,
                          kind="ExternalInput")
    bs_d = nc.dram_tensor("bsides", [128, (GROUPS - 1) * 3 * 3 * 64], f16,
                          kind="ExternalInput")
    xi_d = nc.dram_tensor("xinit", [128, 4 * 2 * COLS], f16,
                          kind="ExternalInput")
    xo_d = nc.dram_tensor("xout", [128, 4 * 2 * COLS], f32,
                          kind="ExternalOutput")

    with tile.TileContext(nc) as tc:
        with (
            tc.tile_pool(name="coef", bufs=1) as cpool,
            tc.tile_pool(name="state", bufs=1) as spool,
            tc.tile_pool(name="psum", bufs=1, space="PSUM") as ppool,
        ):
            bd = cpool.tile([128, (GROUPS - 1) * 4 * 3 * 128], f16, tag="bd")
            bs = cpool.tile([128, (GROUPS - 1) * 3 * 3 * 64], f16, tag="bs")
            xa = spool.tile([128, 4 * 2 * COLS], f16, tag="xa")
            xb = spool.tile([128, 4 * 2 * COLS], f16, tag="xb")
            xout = spool.tile([128, 4 * 2 * COLS], f32, tag="xout")
            nc.sync.dma_start(out=bd[:], in_=bd_d.ap())
            nc.sync.dma_start(out=bs[:], in_=bs_d.ap())
            nc.sync.dma_start(out=xa[:], in_=xi_d.ap())

            def BD(gg, i, var):
                base = ((gg * 4 + i) * 3 + var) * 128
                return bd[:, base:base + 128]

            def BS(gg, t, var):
                base = ((gg * 3 + t) * 3 + var) * 64
                return bs[:, base:base + 64]

            def XCH(t, i, comp):
                return t[:, (i * 2 + comp) * COLS:(i * 2 + comp + 1) * COLS]

            R, I, In = 0, 1, 2
            bufs = [xa, xb]
            for g in range(1, GROUPS):
                gg = g - 1
                src, dst = bufs[gg % 2], bufs[(gg + 1) % 2]
                last = (g == GROUPS - 1)
                for i in range(4):
                    for comp in range(2):
                        ps = ppool.tile([128, COLS], f32, tag=f"ps{i}_{comp}",
                                        name=f"ps{i}_{comp}_{g}")
                        m1 = (R, 0) if comp == 0 else (R, 1)
                        m2 = (In, 1) if comp == 0 else (I, 0)
                        nc.tensor.matmul(
                            out=ps[:], lhsT=BD(gg, i, m1[0]),
                            rhs=XCH(src, i, m1[1]), start=True, stop=False,
                            skip_group_check=True)
                        nc.tensor.matmul(
                            out=ps[:], lhsT=BD(gg, i, m2[0]),
                            rhs=XCH(src, i, m2[1]), start=False, stop=False,
                            skip_group_check=True)
                        mms = []
                        if i < 3:   # dn side j=i+1, boundary t = i
                            for (var, c_in) in ([(R, 0), (In, 1)] if comp == 0
                                                else [(R, 1), (I, 0)]):
                                mms.append((BS(gg, i, var)[0:64, :],
                                            XCH(src, i + 1, c_in)[0:64, :],
                                            ps[64:128, :]))
                        if i > 0:   # up side j=i-1, boundary t = i-1
                            for (var, c_in) in ([(R, 0), (In, 1)] if comp == 0
                                                else [(R, 1), (I, 0)]):
                                mms.append((BS(gg, i - 1, var)[64:128, :],
                                            XCH(src, i - 1, c_in)[64:128, :],
                                            ps[0:64, :]))
                        for k, (lh, rh, po) in enumerate(mms):
                            nc.tensor.matmul(out=po, lhsT=lh, rhs=rh,
                                             start=False,
                                             stop=(k == len(mms) - 1),
                                             skip_group_check=True)
                        out_ap = (XCH(xout, i, comp) if last
                                  else XCH(dst, i, comp))
                        if (i * 2 + comp) % 2 == 0:
                            nc.vector.tensor_copy(out=out_ap, in_=ps[:])
                        else:
                            nc.scalar.copy(out_ap, ps[:])
            nc.sync.dma_start(out=xo_d.ap(), in_=xout[:])
    nc.compile()
    return nc


def _get_modules():
    if "l1" not in _CACHE:
        _CACHE["l1"] = _build_launch1()
        _CACHE["l2"] = _build_launch2()
    return _CACHE["l1"], _CACHE["l2"]


# ---------------------------------------------------------------- entry


def kernel(phases: np.ndarray) -> np.ndarray:
    from concourse.bass_utils import run_bass_kernel_spmd

    phases = np.asarray(phases)
    nc1, nc2 = _get_modules()

    in1 = [_pack_launch1_inputs(phases, g) for g in range(NCORES)]
    res1 = run_bass_kernel_spmd(nc1, in1, core_ids=list(range(NCORES)))
    bts = [_unpack_band(res1.results[g]["vout"]) for g in range(GROUPS)]

    in2 = [_pack_launch2_inputs(bts, c) for c in range(NCORES)]
    res2 = run_bass_kernel_spmd(nc2, in2, core_ids=list(range(NCORES)))

    M = np.zeros((N, N), np.complex64)
    for c in range(NCORES):
        xo = res2.results[c]["xout"]
        cols = slice(c * COLS, (c + 1) * COLS)
        for i in range(4):
            re = xo[:, (2 * i + 0) * COLS:(2 * i + 1) * COLS]
            im = xo[:, (2 * i + 1) * COLS:(2 * i + 2) * COLS]
            M[128 * i:128 * (i + 1), cols] = \
                (re + 1j * im) * np.float32(FINAL_SCALE)
    return M


# revision 23
# speedup vs baseline: 29.2733x; 1.3270x over previous
"""Trainium2 Bass kernel for nn_ClementsBellNxN (N=512, 8 NeuronCores).

Decomposition: the 512 fused 2x2 layers (256 steps x [even, odd]) are split
into 8 groups of 64 layers. Each group's operator B_g is a banded matrix
(half-bandwidth 64). M = D_last . B_7 ... B_0 . D_0.

Launch 1 (SPMD, core g builds BT_g = B_g^T):
  Host fuses each run of 16 layers into a half-bandwidth-16 operator F_s
  (cheap banded numpy; ~4% of device flops). On device, core g computes
  V <- F_s^T . V for s = 4..1 starting from V = I (core 7: D_last; D_0 is
  folded into F_1 of core 0). Block-tridiagonal fp16 PE matmuls with f32
  PSUM accumulation; V stored as 4 row-blocks [128, 264] (absolute column
  windows) x {re, im}.

Host relay: gathers the 8 transposed bands (fp16), chops them into lhsT
  tiles for phase 2, builds each core's initial X = (B_0 D_0)[:, cols].

Launch 2 (SPMD, core c owns 64 columns): X <- B_g . X for g = 1..7 via the
  same block-tridiagonal fp16 PE matmuls (diag [128,128] + corner [64,64]
  triangles), f32 PSUM, fp16 X between groups, f32 out.

Insertion loss scaling: each fused layer carries norm factor 0.95; bands are
rescaled by 0.95^-1 per layer on host and the global 0.95^512 is applied to
the final output to keep everything in fp16 range.
"""
import numpy as np

N = 512
NCORES = 8
GROUPS = 8
SPG = 32          # steps per group
LPG = 64          # fused layers per group
L = 16            # layers per device-fused operator F
NF = LPG // L     # 4 F-operators per group
HB = L            # half-bandwidth of F
WO = 17           # skew window center for host F build
WF = 35           # skew window width for host F build
VW = 264          # on-device V tile column-window width
VB = 68           # V window: block i covers absolute cols [128i-VB, 128i-VB+VW)
COLS = N // NCORES

IL = 0.05
IMB = 0.005
_sq = np.sqrt(1.0 - IL)
A = np.float64(np.float32(_sq * np.sqrt(0.5 + IMB)))
B = np.float64(np.float32(_sq * np.sqrt(0.5 - IMB)))
SC = 1.0 / (1.0 - IL)          # per-fused-layer rescale (|.|^2 norm factor)
FINAL_SCALE = np.float64(1.0 - IL) ** 512

# ---------------------------------------------------------------- host math


def _fused2x2(p1, p2):
    p = np.exp(1j * p1)
    q = np.exp(1j * p2)
    al = A * A * p - B * B * q
    be = 1j * A * B * (p + q)
    de = A * A * q - B * B * p
    return al * SC, be * SC, de * SC


def _shift_m1(X):
    out = np.zeros_like(X)
    out[:, 1:] = X[:, :-1]
    return out


def _shift_p1(X):
    out = np.zeros_like(X)
    out[:, :-1] = X[:, 1:]
    return out


def _apply_even_skew(W, pa):
    # left-multiply skewed band W[r, d] (d = c - r + WO) by Efused(pa)
    k = np.arange(N // 2)
    al, be, de = _fused2x2(pa[2 * k], pa[2 * k + 1])
    T, U = W[0::2], W[1::2]
    nT = al[:, None] * T + be[:, None] * _shift_m1(U)
    nU = be[:, None] * _shift_p1(T) + de[:, None] * U
    W[0::2], W[1::2] = nT, nU


def _apply_odd_skew(W, pb):
    k = np.arange(N // 2 - 1)
    al, be, de = _fused2x2(pb[2 * k + 1], pb[2 * k + 2])
    T, U = W[1:510:2], W[2:511:2]
    nT = al[:, None] * T + be[:, None] * _shift_m1(U)
    nU = be[:, None] * _shift_p1(T) + de[:, None] * U
    W[1:510:2], W[2:511:2] = nT, nU
    W[0] *= np.exp(1j * pb[0]) * SC
    W[511] *= np.exp(1j * pb[511]) * SC


def _group_layer_phases(phases, g):
    """(kind, phase-row) for the 64 layers of group g in application order."""
    out = []
    for i in range(g * SPG, (g + 1) * SPG):
        out.append(('E', np.float64(phases[1 + 2 * i])))
        out.append(('O', np.float64(phases[2 + 2 * i])))
    return out


def _fused_F_dense(phases, g):
    """The NF=4 fused operators of group g as dense [512, 512] complex128."""
    layers = _group_layer_phases(phases, g)
    r = np.arange(N)
    Fs = []
    for s in range(NF):
        W = np.zeros((N, WF), np.complex128)
        W[:, WO] = 1.0
        for kind, ph in layers[s * L:(s + 1) * L]:
            if kind == 'E':
                _apply_even_skew(W, ph)
            else:
                _apply_odd_skew(W, ph)
        F = np.zeros((N, N), np.complex128)
        for dd in range(WF):
            off = dd - WO
            rr = r[(r + off >= 0) & (r + off < N)]
            F[rr, rr + off] = W[rr, dd]
        Fs.append(F)
    if g == 0:
        Fs[0] = Fs[0] * np.exp(1j * np.float64(phases[0]))[None, :]  # F.D0
    return Fs


NDEV = NF - 1     # device applies F_2^T, F_1^T, F_0^T; host pre-applies F_3^T
# pre-step half-bandwidths for the 3 device steps and derived widths
BWS = [16, 32, 48]
CW = [VW] + [128 + 2 * (b + 16) for b in BWS[1:]]        # central mm width
CF0 = [0] + [68 - (b + 16) for b in BWS[1:]]             # central f-start
SW = [16 + 2 * b for b in BWS]                           # side width


def _pack_launch1_inputs(phases, g):
    """fc [128, NDEV*4*3*128], fs [128, NDEV*3*3*64], vinit [128, 4*2*VW]."""
    Fs = _fused_F_dense(phases, g)
    fc = np.zeros((128, NDEV * 4 * 3 * 128), np.float16)
    # side blocks, 64x64 (PE base partitions limited to {0, 32, 64}):
    # partitions 0:64 hold the "dn" block of boundary t (j=t+1 -> out t),
    # partitions 64:128 hold the "up" block (j=t -> out t+1).
    fs = np.zeros((128, NDEV * 3 * 3 * 64), np.float16)
    for s in range(NDEV):
        F = Fs[s]
        for i in range(4):
            blk = F[128 * i:128 * (i + 1), 128 * i:128 * (i + 1)]
            base = ((s * 4 + i) * 3) * 128
            fc[:, base:base + 128] = blk.real.astype(np.float16)
            fc[:, base + 128:base + 256] = blk.imag.astype(np.float16)
            fc[:, base + 256:base + 384] = (-blk.imag).astype(np.float16)
        for t in range(3):
            dn = F[128 * (t + 1):128 * (t + 1) + 64,
                   128 * (t + 1) - 64:128 * (t + 1)]
            up = F[128 * t + 64:128 * (t + 1),
                   128 * (t + 1):128 * (t + 1) + 64]
            base = ((s * 3 + t) * 3) * 64
            fs[0:64, base:base + 64] = dn.real.astype(np.float16)
            fs[0:64, base + 64:base + 128] = dn.imag.astype(np.float16)
            fs[0:64, base + 128:base + 192] = (-dn.imag).astype(np.float16)
            fs[64:128, base:base + 64] = up.real.astype(np.float16)
            fs[64:128, base + 64:base + 128] = up.imag.astype(np.float16)
            fs[64:128, base + 128:base + 192] = (-up.imag).astype(np.float16)
    # host pre-applies the first device factor: V_1 = F_3^T . D
    if g == GROUPS - 1:
        d = np.exp(1j * np.float64(phases[N + 1]))
    else:
        d = np.ones(N)
    V1 = (Fs[NF - 1] * d[:, None]).T        # (D.F_3)^T, half-bandwidth 16
    vinit = np.zeros((128, 4 * 2 * VW), np.float16)
    for i in range(4):
        lo = 128 * i - VB
        c0, c1 = max(0, lo), min(N, lo + VW)
        blk = V1[128 * i:128 * (i + 1), c0:c1]
        vinit[:, (i * 2 + 0) * VW + (c0 - lo):(i * 2 + 0) * VW + (c1 - lo)] = \
            blk.real.astype(np.float16)
        vinit[:, (i * 2 + 1) * VW + (c0 - lo):(i * 2 + 1) * VW + (c1 - lo)] = \
            blk.imag.astype(np.float16)
    return {"fc": fc, "fs": fs, "vinit": vinit}


def _unpack_band(vout):
    """vout [128, 4*2*VW] f16 -> BT dense ([512,512] f16 re, im)."""
    btr = np.zeros((N, N), np.float16)
    bti = np.zeros((N, N), np.float16)
    for i in range(4):
        lo = 128 * i - VB
        c0, c1 = max(0, lo), min(N, lo + VW)
        btr[128 * i:128 * (i + 1), c0:c1] = \
            vout[:, (i * 2 + 0) * VW + (c0 - lo):(i * 2 + 0) * VW + (c1 - lo)]
        bti[128 * i:128 * (i + 1), c0:c1] = \
            vout[:, (i * 2 + 1) * VW + (c0 - lo):(i * 2 + 1) * VW + (c1 - lo)]
    return btr, bti


def _pack_launch2_inputs(bts, core):
    """bands [128, 7*4*3*128] diag + bsides [64, 7*6*3*64] + xinit."""
    bd = np.zeros((128, (GROUPS - 1) * 4 * 3 * 128), np.float16)
    # side blocks, 64x64: partitions 0:64 = dn (j=t+1 -> out t), partitions
    # 64:128 = up (j=t -> out t+1). One 64-col slot per (group, boundary, var).
    bs = np.zeros((128, (GROUPS - 1) * 3 * 3 * 64), np.float16)
    for g in range(1, GROUPS):
        btr, bti = bts[g]
        gg = g - 1
        for i in range(4):
            base = ((gg * 4 + i) * 3) * 128
            r0 = 128 * i
            bd[:, base:base + 128] = btr[r0:r0 + 128, r0:r0 + 128]
            bd[:, base + 128:base + 256] = bti[r0:r0 + 128, r0:r0 + 128]
            bd[:, base + 256:base + 384] = -bti[r0:r0 + 128, r0:r0 + 128]
        for t in range(3):
            qd, md = 128 * (t + 1), 128 * t + 64     # dn: BT[qd:+64, md:+64]
            qu, mu = 128 * t + 64, 128 * (t + 1)     # up: BT[qu:+64, mu:+64]
            base = ((gg * 3 + t) * 3) * 64
            bs[0:64, base:base + 64] = btr[qd:qd + 64, md:md + 64]
            bs[0:64, base + 64:base + 128] = bti[qd:qd + 64, md:md + 64]
            bs[0:64, base + 128:base + 192] = -bti[qd:qd + 64, md:md + 64]
            bs[64:128, base:base + 64] = btr[qu:qu + 64, mu:mu + 64]
            bs[64:128, base + 64:base + 128] = bti[qu:qu + 64, mu:mu + 64]
            bs[64:128, base + 128:base + 192] = -bti[qu:qu + 64, mu:mu + 64]
    btr0, bti0 = bts[0]
    cols = slice(core * COLS, (core + 1) * COLS)
    xinit = np.zeros((128, 4 * 2 * COLS), np.float16)
    for i in range(4):
        xinit[:, (2 * i + 0) * COLS:(2 * i + 1) * COLS] = \
            btr0[cols, 128 * i:128 * (i + 1)].T
        xinit[:, (2 * i + 1) * COLS:(2 * i + 2) * COLS] = \
            bti0[cols, 128 * i:128 * (i + 1)].T
    return {"bands": bd, "bsides": bs, "xinit": xinit}


# ---------------------------------------------------------------- bass build

_CACHE = {}


def _build_launch1():
    import concourse.mybir as mybir
    from concourse import bacc, tile

    f16 = mybir.dt.float16
    f32 = mybir.dt.float32

    nc = bacc.Bacc("TRN2", target_bir_lowering=False, debug=False,
                   enable_asserts=False)
    fc_d = nc.dram_tensor("fc", [128, NDEV * 4 * 3 * 128], f16,
                          kind="ExternalInput")
    fs_d = nc.dram_tensor("fs", [128, NDEV * 3 * 3 * 64], f16,
                          kind="ExternalInput")
    vi_d = nc.dram_tensor("vinit", [128, 4 * 2 * VW], f16,
                          kind="ExternalInput")
    vo_d = nc.dram_tensor("vout", [128, 4 * 2 * VW], f16,
                          kind="ExternalOutput")

    with tile.TileContext(nc) as tc:
        with (
            tc.tile_pool(name="coef", bufs=1) as cpool,
            tc.tile_pool(name="state", bufs=1) as spool,
            tc.tile_pool(name="psum", bufs=1, space="PSUM") as ppool,
        ):
            fc = cpool.tile([128, NDEV * 4 * 3 * 128], f16, tag="fc")
            fs = cpool.tile([128, NDEV * 3 * 3 * 64], f16, tag="fs")
            va = spool.tile([128, 4 * 2 * VW], f16, tag="va")
            vb = spool.tile([128, 4 * 2 * VW], f16, tag="vb")
            # split input DMAs in use order: vinit first, then per-step F
            # chunks (device applies s = NDEV-1 .. 0)
            nc.sync.dma_start(out=va[:], in_=vi_d.ap())
            for s in reversed(range(NDEV)):
                c0, c1 = (s * 4) * 3 * 128, ((s + 1) * 4) * 3 * 128
                nc.sync.dma_start(out=fc[:, c0:c1], in_=fc_d.ap()[:, c0:c1])
                c0, c1 = (s * 3) * 3 * 64, ((s + 1) * 3) * 3 * 64
                nc.sync.dma_start(out=fs[:, c0:c1], in_=fs_d.ap()[:, c0:c1])

            def FC(s, i, var):
                base = ((s * 4 + i) * 3 + var) * 128
                return fc[:, base:base + 128]

            def FS(s, t, var):
                base = ((s * 3 + t) * 3 + var) * 64
                return fs[:, base:base + 64]

            def VCH(t, i, comp, f0=0, f1=VW):
                b = (i * 2 + comp) * VW
                return t[:, b + f0:b + f1]

            R, I, In = 0, 1, 2
            bufs = [va, vb]
            for sidx, s in enumerate(reversed(range(NDEV))):
                src, dst = bufs[sidx % 2], bufs[(sidx + 1) % 2]
                b = BWS[sidx]
                cf0, cw = CF0[sidx], CW[sidx]      # central slice
                sw = SW[sidx]                      # side width
                dn0, up0 = 68 - b, 180 - b         # side f-starts (src coords)
                for i in range(4):
                    for comp in range(2):
                        ps = ppool.tile([128, VW], f32, tag=f"ps{i}_{comp}",
                                        name=f"ps{i}_{comp}_{s}")
                        # central: out_R = FR.Vr - FI.Vi ; out_I = FR.Vi+FI.Vr
                        m1 = (R, 0) if comp == 0 else (R, 1)
                        m2 = (In, 1) if comp == 0 else (I, 0)
                        nc.tensor.matmul(
                            out=ps[:, cf0:cf0 + cw], lhsT=FC(s, i, m1[0]),
                            rhs=VCH(src, i, m1[1], cf0, cf0 + cw),
                            start=True, stop=False, skip_group_check=True)
                        nc.tensor.matmul(
                            out=ps[:, cf0:cf0 + cw], lhsT=FC(s, i, m2[0]),
                            rhs=VCH(src, i, m2[1], cf0, cf0 + cw),
                            start=False, stop=False, skip_group_check=True)
                        mms = []
                        if i < 3:   # dn side: j = i+1, boundary t = i
                            for (var, c_in) in ([(R, 0), (In, 1)] if comp == 0
                                                else [(R, 1), (I, 0)]):
                                mms.append((FS(s, i, var)[0:64, :],
                                            VCH(src, i + 1, c_in,
                                                dn0, dn0 + sw)[0:64, :],
                                            ps[64:128, dn0 + 128:
                                               dn0 + 128 + sw]))
                        if i > 0:   # up side: j = i-1, boundary t = i-1
                            for (var, c_in) in ([(R, 0), (In, 1)] if comp == 0
                                                else [(R, 1), (I, 0)]):
                                mms.append((FS(s, i - 1, var)[64:128, :],
                                            VCH(src, i - 1, c_in,
                                                up0, up0 + sw)[64:128, :],
                                            ps[0:64, up0 - 128:
                                               up0 - 128 + sw]))
                        for k, (lh, rh, po) in enumerate(mms):
                            nc.tensor.matmul(out=po, lhsT=lh, rhs=rh,
                                             start=False,
                                             stop=(k == len(mms) - 1),
                                             skip_group_check=True)
                        # evacuate PSUM -> dst (f16)
                        if (i * 2 + comp) % 2 == 0:
                            nc.vector.tensor_copy(
                                out=VCH(dst, i, comp, cf0, cf0 + cw),
                                in_=ps[:, cf0:cf0 + cw])
                        else:
                            nc.scalar.copy(VCH(dst, i, comp, cf0, cf0 + cw),
                                           ps[:, cf0:cf0 + cw])
            final = bufs[NDEV % 2]
            nc.sync.dma_start(out=vo_d.ap(), in_=final[:])
    nc.compile()
    return nc


def _build_launch2():
    import concourse.mybir as mybir
    from concourse import bacc, tile

    f16 = mybir.dt.float16
    f32 = mybir.dt.float32

    nc = bacc.Bacc("TRN2", target_bir_lowering=False, debug=False,
                   enable_asserts=False)
    bd_d = nc.dram_tensor("bands", [128, (GROUPS - 1) * 4 * 3 * 128], f16,
                          kind="ExternalInput")
    bs_d = nc.dram_tensor("bsides", [128, (GROUPS - 1) * 3 * 3 * 64], f16,
                          kind="ExternalInput")
    xi_d = nc.dram_tensor("xinit", [128, 4 * 2 * COLS], f16,
                          kind="ExternalInput")
    xo_d = nc.dram_tensor("xout", [128, 4 * 2 * COLS], f32,
                          kind="ExternalOutput")

    with tile.TileContext(nc) as tc:
        with (
            tc.tile_pool(name="coef", bufs=1) as cpool,
            tc.tile_pool(name="state", bufs=1) as spool,
            tc.tile_pool(name="psum", bufs=1, space="PSUM") as ppool,
        ):
            bd = cpool.tile([128, (GROUPS - 1) * 4 * 3 * 128], f16, tag="bd")
            bs = cpool.tile([128, (GROUPS - 1) * 3 * 3 * 64], f16, tag="bs")
            xa = spool.tile([128, 4 * 2 * COLS], f16, tag="xa")
            xb = spool.tile([128, 4 * 2 * COLS], f16, tag="xb")
            xout = spool.tile([128, 4 * 2 * COLS], f32, tag="xout")
            # xinit first, then per-group band tiles in use order
            nc.sync.dma_start(out=xa[:], in_=xi_d.ap())
            for gg in range(GROUPS - 1):
                c0, c1 = (gg * 4) * 3 * 128, ((gg + 1) * 4) * 3 * 128
                nc.sync.dma_start(out=bd[:, c0:c1], in_=bd_d.ap()[:, c0:c1])
                c0, c1 = (gg * 3) * 3 * 64, ((gg + 1) * 3) * 3 * 64
                nc.sync.dma_start(out=bs[:, c0:c1], in_=bs_d.ap()[:, c0:c1])

            def BD(gg, i, var):
                base = ((gg * 4 + i) * 3 + var) * 128
                return bd[:, base:base + 128]

            def BS(gg, t, var):
                base = ((gg * 3 + t) * 3 + var) * 64
                return bs[:, base:base + 64]

            def XCH(t, i, comp):
                return t[:, (i * 2 + comp) * COLS:(i * 2 + comp + 1) * COLS]

            R, I, In = 0, 1, 2
            bufs = [xa, xb]
            for g in range(1, GROUPS):
                gg = g - 1
                src, dst = bufs[gg % 2], bufs[(gg + 1) % 2]
                last = (g == GROUPS - 1)
                for i in range(4):
                    for comp in range(2):
                        ps = ppool.tile([128, COLS], f32, tag=f"ps{i}_{comp}",
                                        name=f"ps{i}_{comp}_{g}")
                        m1 = (R, 0) if comp == 0 else (R, 1)
                        m2 = (In, 1) if comp == 0 else (I, 0)
                        nc.tensor.matmul(
                            out=ps[:], lhsT=BD(gg, i, m1[0]),
                            rhs=XCH(src, i, m1[1]), start=True, stop=False,
                            skip_group_check=True)
                        nc.tensor.matmul(
                            out=ps[:], lhsT=BD(gg, i, m2[0]),
                            rhs=XCH(src, i, m2[1]), start=False, stop=False,
                            skip_group_check=True)
                        mms = []
                        if i < 3:   # dn side j=i+1, boundary t = i
                            for (var, c_in) in ([(R, 0), (In, 1)] if comp == 0
                                                else [(R, 1), (I, 0)]):
                                mms.append((BS(gg, i, var)[0:64, :],
                                            XCH(src, i + 1, c_in)[0:64, :],
                                            ps[64:128, :]))
                        if i > 0:   # up side j=i-1, boundary t = i-1
                            for (var, c_in) in ([(R, 0), (In, 1)] if comp == 0
                                                else [(R, 1), (I, 0)]):
                                mms.append((BS(gg, i - 1, var)[64:128, :],
                                            XCH(src, i - 1, c_in)[64:128, :],
                                            ps[0:64, :]))
                        for k, (lh, rh, po) in enumerate(mms):
                            nc.tensor.matmul(out=po, lhsT=lh, rhs=rh,
                                             start=False,
                                             stop=(k == len(mms) - 1),
                                             skip_group_check=True)
                        out_ap = (XCH(xout, i, comp) if last
                                  else XCH(dst, i, comp))
                        if (i * 2 + comp) % 2 == 0:
                            nc.vector.tensor_copy(out=out_ap, in_=ps[:])
                        else:
                            nc.scalar.copy(out_ap, ps[:])
            nc.sync.dma_start(out=xo_d.ap(), in_=xout[:])
    nc.compile()
    return nc


def _get_modules():
    if "l1" not in _CACHE:
        _CACHE["l1"] = _build_launch1()
        _CACHE["l2"] = _build_launch2()
    return _CACHE["l1"], _CACHE["l2"]


# ---------------------------------------------------------------- entry


def kernel(phases: np.ndarray) -> np.ndarray:
    from concourse.bass_utils import run_bass_kernel_spmd

    phases = np.asarray(phases)
    nc1, nc2 = _get_modules()

    in1 = [_pack_launch1_inputs(phases, g) for g in range(NCORES)]
    res1 = run_bass_kernel_spmd(nc1, in1, core_ids=list(range(NCORES)))
    bts = [_unpack_band(res1.results[g]["vout"]) for g in range(GROUPS)]

    in2 = [_pack_launch2_inputs(bts, c) for c in range(NCORES)]
    res2 = run_bass_kernel_spmd(nc2, in2, core_ids=list(range(NCORES)))

    M = np.zeros((N, N), np.complex64)
    for c in range(NCORES):
        xo = res2.results[c]["xout"]
        cols = slice(c * COLS, (c + 1) * COLS)
        for i in range(4):
            re = xo[:, (2 * i + 0) * COLS:(2 * i + 1) * COLS]
            im = xo[:, (2 * i + 1) * COLS:(2 * i + 2) * COLS]
            M[128 * i:128 * (i + 1), cols] = \
                (re + 1j * im) * np.float32(FINAL_SCALE)
    return M
